# revision 62
# baseline (speedup 1.0000x reference)
"""Block-local self-attention (BigBird-style window + one global token) on 8
Trainium2 NeuronCores.

Problem (hardcoded): n=2, h=16, t=4096, d=64, block=128, fp32 in/out,
attention_mask all-zeros.  Per (n,h) pair, query block g attends to K/V
positions [128(g-1), 128(g+2)) plus the global token 0; query 0 attends to all
4096 positions.

Sharding: pure data parallel - the 32 (n,h) pairs split 4 per core; no
collectives.

Device does ONLY the three big streams per pair:
  - QK: S^T per 128-token K-chunk j into [128, 2, 512] PSUM tiles, fp16.
    The stationary is the couple-stacked [K_A^T; K_B^T] (128 rows) and the
    moving operand is a BLOCK-DIAGONAL qt copy ([Q_A^T; 0] or [0; Q_B^T]):
    128-partition moving data streams fp16 at 2 cols/cycle, twice the rate
    of the naive 64-partition [d, T] layout, and the zero half kills the
    cross-pair terms.
  - exp on ACT per 2 chunks (768 cols amortizes the ACT access latency),
    fp16 out.  No masking: the kpos-0 "local copy" weight for query blocks
    0-1 equals the reference's global-column weight exp(q.K0), so it is
    kept.  ACT is the only engine with exp and is the ~50us/core floor.
  - AV out^T accumulated per 512-query PSUM bank: first writer start=True
    zeroes the whole 2KB bank (ZERO_REGION), the rest accumulate; V ships
    kpos-major with a ones column so Z rides row 64.  Two banks' writers
    are interleaved so consecutive matmuls hit different PSUM banks (the
    accumulator read-modify-write otherwise serializes).  Eviction
    PSUM->SBUF fp16 on DVE, then a per-bank store drains continuously.
AV bank-pairs are woven into the QK group stream a few groups behind their
exp dependencies (cross-pair), so the PE never stalls on ACT mid-stream;
the last pair emits single banks greedily to minimize the tail.

Host finishing (cheap, O(t) or O(t*d) numpy): adds the global-token rank-1
term e_g (x) [v0|1] for queries >= 256 (blocks 0-1 already got kpos 0 via
their window), normalizes by Z, computes the global-query row 0 exactly, and
transposes back to [t, d].
"""

import numpy as np

import concourse.bass as bass
import concourse.bacc as bacc
import concourse.tile as tile
from concourse import mybir
from concourse.bass_utils import run_bass_kernel_spmd

# ---- problem constants ----
N, H, T, D = 2, 16, 4096, 64
B = 128
NB = T // B            # 32 blocks
NAUG = D + 1           # V with ones column
NCORES = 8
NPAIR = (N * H) // NCORES   # 4 pairs per core
SCALE = 1.0 / np.sqrt(D)
BANKQ = 512            # query columns per out^T PSUM bank
NBANK = T // BANKQ     # 8

QK_DT = mybir.dt.float16
AV_DT = mybir.dt.float16
F32 = mybir.dt.float32


def _chunk_q0(j):
    return B * max(j - 1, 0)


def _chunk_q1(j):
    return min(B * (j + 2), T)


def _bank_writers():
    writers = [[] for _ in range(NBANK)]
    for j in range(NB):
        a, q1 = _chunk_q0(j), _chunk_q1(j)
        while a < q1:
            nxt = min(q1, (a // BANKQ + 1) * BANKQ)
            writers[a // BANKQ].append((j, a, nxt))
            a = nxt
    return writers


def build_nc(npair=NPAIR):
    nc = bacc.Bacc("TRN2", target_bir_lowering=False, debug=False)
    ncoup = npair // 2

    # qt is block-diagonal per couple: copy 0 = [Q_A^T; 0], copy 1 = [0; Q_B^T]
    # (partition-major layout [128, 2, T]).  The QK matmul then runs with
    # 128-partition moving data, which streams fp16 at 2 cols/cycle - twice
    # the rate of a 64-partition moving operand.  The couple-stacked kt
    # [K_A^T; K_B^T] is the shared 128-row stationary; the zero half of qt
    # kills the cross-pair terms.
    # copy-major qt: each [128, T] copy is a sequential DRAM read.  sg0..sg5
    # hold couple-0's critical kt/qt segments as separate fully-contiguous
    # tensors: column-sliced loads of the row-major tensors read 1-2KB rows
    # at 8KB stride and crawl at ~21GB/s, so every head-path DMA sources
    # from a contiguous block instead.
    # only the NONZERO 64 rows of each block-diagonal qt copy ship from
    # DRAM; the zero halves are memset on the otherwise-idle DVE.  That
    # halves the latency-critical input bytes.
    qt_d = nc.dram_tensor("qt", [ncoup, 2, D, T], QK_DT, kind="ExternalInput").ap()
    kt_d = nc.dram_tensor("kt", [ncoup, 2 * D, T], QK_DT, kind="ExternalInput").ap()
    sg_shapes = ((2 * D, 512), (D, 512), (2 * D, 1024), (D, 1536),
                 (2 * D, 1280), (2 * D, 1280), (D, 2048))
    sg_d = []
    for i, shp in enumerate(sg_shapes):
        sg_d.append(nc.dram_tensor(f"sg{i}", list(shp), QK_DT,
                                   kind="ExternalInput").ap())
    va_d = nc.dram_tensor("va", [npair, B, NB * NAUG], AV_DT, kind="ExternalInput").ap()
    # unnormalized transposed output + Z row: [65, nbank, 512] fp16 per pair
    oz_d = nc.dram_tensor("oz", [npair, NAUG, NBANK * BANKQ], AV_DT,
                          kind="ExternalOutput").ap()

    Exp = mybir.ActivationFunctionType.Exp
    writers = _bank_writers()

    with tile.TileContext(nc) as tc:
        with (
            tc.tile_pool(name="qk", bufs=2) as qk_pool,
            tc.tile_pool(name="v", bufs=4) as v_pool,
            tc.tile_pool(name="e", bufs=2) as e_pool,
            tc.tile_pool(name="out", bufs=2) as out_pool,
            tc.tile_pool(name="qkps", bufs=2, space="PSUM") as qk_psum,
            tc.tile_pool(name="avps", bufs=4, space="PSUM") as av_psum,
        ):
            # ---- prologue: all input loads up front ----
            qts, kts, vas = [], [], []
            FQ = 512   # head segment: covers the first QK groups
            MQ = 1536
            for c in range(ncoup):
                qt_sb = qk_pool.tile([2 * D, 2, T], QK_DT, tag="qt")
                kt_sb = qk_pool.tile([2 * D, T], QK_DT, tag="kt")
                qts.append(qt_sb)
                kts.append(kt_sb)
            for ip in range(npair):
                va_sb = v_pool.tile([B, NB, NAUG], AV_DT, tag="va", name="va")
                vas.append(va_sb)
            # sync and scalar are HARDWARE DGE queues (fast); gpsimd DMA
            # is software-executed on the Q7 and ~2x slower, so couple-0's
            # critical stream rides sync/scalar from the contiguous sg
            # blocks and late-need bulk rides gpsimd as sequential full-copy
            # reads.
            # zero halves of the four qt copies (DVE is idle here)
            nc.vector.memset(qts[0][D:2 * D, 0, :], 0.0)
            nc.vector.memset(qts[0][0:D, 1, :], 0.0)
            nc.vector.memset(qts[1][D:2 * D, 0, :], 0.0)
            nc.vector.memset(qts[1][0:D, 1, :], 0.0)
            nc.sync.dma_start(out=kts[0][:, 0:FQ], in_=sg_d[0])
            nc.scalar.dma_start(out=qts[0][0:D, 0, 0:FQ], in_=sg_d[1])
            nc.sync.dma_start(out=kts[0][:, FQ:MQ], in_=sg_d[2])
            nc.scalar.dma_start(out=qts[0][0:D, 0, FQ:2048], in_=sg_d[3])
            nc.scalar.dma_start(out=qts[0][0:D, 0, 2048:T], in_=sg_d[6])
            nc.sync.dma_start(out=kts[0][:, MQ:2816], in_=sg_d[4])
            nc.scalar.dma_start(out=kts[0][:, 2816:T], in_=sg_d[5])
            nc.gpsimd.dma_start(out=vas[0], in_=va_d[0])
            nc.gpsimd.dma_start(out=qts[0][D:2 * D, 1, :], in_=qt_d[0, 1])
            nc.sync.dma_start(out=kts[1][:, 0:T], in_=kt_d[1])
            nc.gpsimd.dma_start(out=vas[1], in_=va_d[1])
            nc.gpsimd.dma_start(out=qts[1][0:D, 0, :], in_=qt_d[1, 0])
            nc.gpsimd.dma_start(out=qts[1][D:2 * D, 1, :], in_=qt_d[1, 1])
            nc.gpsimd.dma_start(out=vas[2], in_=va_d[2])
            nc.sync.dma_start(out=vas[3], in_=va_d[3])

            exps = [None] * npair
            osbs = [None] * npair

            # ---------- per-pair unit streams ----------
            def qk_group(ip, g):
                c, hh = ip // 2, ip % 2
                qt_sb, kt_sb = qts[c], kts[c]

                def run():
                    if g == 0:
                        exps[ip] = e_pool.tile([B, NB, 3 * B], AV_DT, tag="exp",
                                               name="exp")
                        osbs[ip] = out_pool.tile([NAUG, NBANK, BANKQ], AV_DT,
                                                 tag="osb", name="osb")
                    ps = qk_psum.tile([B, 2, BANKQ], F32, tag="qkps")
                    for ti in range(2):
                        j = 2 * g + ti
                        # uniform 384-wide window (edge chunks widened so
                        # every exp call is one full batch)
                        q0w = min(_chunk_q0(j), T - 3 * B)
                        nc.tensor.matmul(
                            ps[:, ti, 0:3 * B],
                            lhsT=kt_sb[:, j * B:(j + 1) * B],
                            rhs=qt_sb[:, hh, q0w:q0w + 3 * B],
                            start=True,
                            stop=True,
                        )
                    nc.scalar.activation(
                        out=exps[ip][:, 2 * g:2 * g + 2, :],
                        in_=ps[:, :, 0:3 * B],
                        func=Exp, scale=float(SCALE),
                    )
                return run

            def av_banks(ip, blist):
                # emit the banks' writers interleaved: consecutive matmuls
                # then hit DIFFERENT psum banks, so the accumulator
                # read-modify-write of one bank overlaps the other's stream
                def run():
                    exp_sb = exps[ip]
                    va_sb = vas[ip]
                    avs, wls = [], []
                    for b in blist:
                        avs.append(av_psum.tile([NAUG, BANKQ], F32, tag="avps",
                                                name="avtile"))
                        wls.append(list(writers[b]))
                    for wi in range(max(len(w) for w in wls)):
                        for k, b in enumerate(blist):
                            if wi >= len(wls[k]):
                                continue
                            j, a0, a1 = wls[k][wi]
                            q0w = min(_chunk_q0(j), T - 3 * B)
                            nc.tensor.matmul(
                                avs[k][:, a0 - BANKQ * b:a1 - BANKQ * b],
                                lhsT=va_sb[:, j, :],
                                rhs=exp_sb[:, j, a0 - q0w:a1 - q0w],
                                start=(wi == 0),  # zeroes the whole 2KB bank
                                stop=(wi == len(wls[k]) - 1),
                                skip_group_check=(wi != 0),
                            )
                    for k, b in enumerate(blist):
                        # eviction on DVE (gpsimd cannot access PSUM), then a
                        # per-bank store so the output drains continuously.
                        # The very last bank evicts on ACT (idle after its
                        # final exp) so the tail's two evictions run in
                        # parallel instead of serializing on DVE.
                        if ip == npair - 1 and b == NBANK - 1:
                            nc.scalar.activation(
                                out=osbs[ip][:, b, :], in_=avs[k],
                                func=mybir.ActivationFunctionType.Copy)
                        else:
                            nc.vector.tensor_copy(out=osbs[ip][:, b, :],
                                                  in_=avs[k])
                        # all stores on the hardware sync queue: exec time
                        # ends at the last DMA + queue drains, and the
                        # software gpsimd queue both transfers slowly and
                        # drains slowly when it still has pending stores
                        seng = nc.sync
                        seng.dma_start(
                            out=oz_d[ip, :, b * BANKQ:(b + 1) * BANKQ],
                            in_=osbs[ip][:, b, :],
                        )
                return run

            # ---------- emission: AV banks woven into the QK stream ----------
            # bank (p, b) consumes chunks up to 4b+6, i.e. QK group 2b+3 of
            # pair p.  Emit it SLACK groups later so the exp it needs is
            # already drained from ACT and the PE never stalls mid-stream;
            # late banks spill into the next pair's groups.
            # bank-pair (b0, b0+1) consumes chunks up to 4(b0+1)+6, i.e. QK
            # group 2*b0+5 of its pair.  The first pair runs with less slack
            # (no AV backlog exists yet to fill PE waits anyway) and the
            # last pair emits single banks as soon as their exps exist so
            # the tail after the final QK group is one bank, not four.
            NG = NB // 2
            av_ready = []
            for p in range(npair):
                slack = 2 if p == 0 else (3 if p < npair - 1 else 1)
                if p < npair - 1:
                    for b0 in range(0, NBANK, 2):
                        av_ready.append(
                            (NG * p + min(2 * b0 + 5, NG - 1) + slack,
                             p, (b0, b0 + 1)))
                else:
                    for b in range(NBANK):
                        av_ready.append(
                            (NG * p + min(2 * b + 3, NG - 1) + slack,
                             p, (b,)))
            av_ready.sort(key=lambda t: t[0])
            ai = 0
            for gi in range(npair * NG):
                qk_group(gi // NG, gi % NG)()
                while ai < len(av_ready) and av_ready[ai][0] <= gi:
                    _, p, blist = av_ready[ai]
                    av_banks(p, blist)()
                    ai += 1
            while ai < len(av_ready):
                _, p, blist = av_ready[ai]
                av_banks(p, blist)()
                ai += 1

    nc.compile()
    return nc


_CACHE = {}


def _prep_core(q, k, v, core):
    sl = slice(core * NPAIR, (core + 1) * NPAIR)
    np_qk = mybir.dt.np(QK_DT)
    qs, ks, vs = q[sl], k[sl], v[sl]
    ncoup = NPAIR // 2
    # kt: [ncoup, 2D, T] - two pairs of a couple stacked on partitions
    # qt: [ncoup, 2D, 2, T] block-diagonal: [:, 0:64, 0, :] = Q_A^T,
    #     [:, 64:128, 1, :] = Q_B^T, rest zeros
    # nonzero halves only: copy 0 = Q_A^T rows, copy 1 = Q_B^T rows
    qt = np.ascontiguousarray(
        qs.reshape(ncoup, 2, T, D).transpose(0, 1, 3, 2).astype(np_qk))
    kt = np.ascontiguousarray(
        ks.reshape(ncoup, 2, T, D).transpose(0, 1, 3, 2)
        .reshape(ncoup, 2 * D, T).astype(np_qk))
    # va: [npair, B, NB*NAUG] kpos-major with ones column
    va = np.concatenate([vs, np.ones((NPAIR, T, 1), np.float32)], axis=-1)
    va = va.reshape(NPAIR, NB, B, NAUG).transpose(0, 2, 1, 3)
    va = np.ascontiguousarray(
        va.reshape(NPAIR, B, NB * NAUG).astype(mybir.dt.np(AV_DT))
    )
    out = {"qt": qt, "kt": kt, "va": va}
    for i, (src_t, c0, c1) in enumerate((
            (kt[0], 0, 512), (qt[0, 0], 0, 512),
            (kt[0], 512, 1536), (qt[0, 0], 512, 2048),
            (kt[0], 1536, 2816), (kt[0], 2816, 4096),
            (qt[0, 0], 2048, 4096))):
        out[f"sg{i}"] = np.ascontiguousarray(src_t[:, c0:c1])
    return out


def kernel(query_layer, key_layer, value_layer, attention_mask):
    q = np.asarray(query_layer, np.float32).reshape(N * H, T, D)
    k = np.asarray(key_layer, np.float32).reshape(N * H, T, D)
    v = np.asarray(value_layer, np.float32).reshape(N * H, T, D)

    if "nc" not in _CACHE:
        _CACHE["nc"] = build_nc()
    nc = _CACHE["nc"]

    in_maps = [_prep_core(q, k, v, core) for core in range(NCORES)]
    res = run_bass_kernel_spmd(nc, in_maps, core_ids=list(range(NCORES)))
    # [NCORES, NPAIR, 65, NBANK*BANKQ] fp16 -> [32, 65, 4096] f32
    oz = np.stack([r["oz"] for r in res.results]).astype(np.float32)
    oz = oz.reshape(N * H, NAUG, T)
    o_un = oz[:, 0:D, :]              # [32, 64, 4096] unnormalized out^T
    z = oz[:, D, :]                   # [32, 4096]

    # global-token rank-1 term for queries >= 2 blocks (blocks 0-1 already
    # include kpos 0 through their local window)
    eg = np.exp(np.einsum('ptd,pd->pt', q, k[:, 0]) * SCALE)  # [32, 4096]
    o_un[:, :, 2 * B:] += eg[:, None, 2 * B:] * v[:, 0, :, None]
    z[:, 2 * B:] += eg[:, 2 * B:]

    out = (o_un / z[:, None, :]).transpose(0, 2, 1)  # [32, 4096, 64]

    # global query row: exact softmax over all positions
    p0 = np.exp(np.einsum('pd,ptd->pt', q[:, 0], k) * SCALE)
    out[:, 0, :] = np.einsum('pt,ptd->pd', p0, v) / p0.sum(1)[:, None]

    return np.ascontiguousarray(out.reshape(N, H, T, D).astype(np.float32))


# revision 63
# speedup vs baseline: 1.0274x; 1.0274x over previous
"""Block-local self-attention (BigBird-style window + one global token) on 8
Trainium2 NeuronCores.

Problem (hardcoded): n=2, h=16, t=4096, d=64, block=128, fp32 in/out,
attention_mask all-zeros.  Per (n,h) pair, query block g attends to K/V
positions [128(g-1), 128(g+2)) plus the global token 0; query 0 attends to all
4096 positions.

Sharding: pure data parallel - the 32 (n,h) pairs split 4 per core; no
collectives.

Device does ONLY the three big streams per pair:
  - QK: S^T per 128-token K-chunk j into [128, 2, 512] PSUM tiles, fp16.
    The stationary is the couple-stacked [K_A^T; K_B^T] (128 rows) and the
    moving operand is a BLOCK-DIAGONAL qt copy ([Q_A^T; 0] or [0; Q_B^T]):
    128-partition moving data streams fp16 at 2 cols/cycle, twice the rate
    of the naive 64-partition [d, T] layout, and the zero half kills the
    cross-pair terms.
  - exp on ACT per 2 chunks (768 cols amortizes the ACT access latency),
    fp16 out.  No masking: the kpos-0 "local copy" weight for query blocks
    0-1 equals the reference's global-column weight exp(q.K0), so it is
    kept.  ACT is the only engine with exp and is the ~50us/core floor.
  - AV out^T accumulated per 512-query PSUM bank: first writer start=True
    zeroes the whole 2KB bank (ZERO_REGION), the rest accumulate; V ships
    kpos-major with a ones column so Z rides row 64.  Two banks' writers
    are interleaved so consecutive matmuls hit different PSUM banks (the
    accumulator read-modify-write otherwise serializes).  Eviction
    PSUM->SBUF fp16 on DVE, then a per-bank store drains continuously.
AV bank-pairs are woven into the QK group stream a few groups behind their
exp dependencies (cross-pair), so the PE never stalls on ACT mid-stream;
the last pair emits single banks greedily to minimize the tail.

Host finishing (cheap, O(t) or O(t*d) numpy): adds the global-token rank-1
term e_g (x) [v0|1] for queries >= 256 (blocks 0-1 already got kpos 0 via
their window), normalizes by Z, computes the global-query row 0 exactly, and
transposes back to [t, d].
"""

import numpy as np

import concourse.bass as bass
import concourse.bacc as bacc
import concourse.tile as tile
from concourse import mybir
from concourse.bass_utils import run_bass_kernel_spmd

# ---- problem constants ----
N, H, T, D = 2, 16, 4096, 64
B = 128
NB = T // B            # 32 blocks
NAUG = D + 1           # V with ones column
NCORES = 8
NPAIR = (N * H) // NCORES   # 4 pairs per core
SCALE = 1.0 / np.sqrt(D)
BANKQ = 512            # query columns per out^T PSUM bank
NBANK = T // BANKQ     # 8

QK_DT = mybir.dt.float16
AV_DT = mybir.dt.float16
F32 = mybir.dt.float32


def _chunk_q0(j):
    return B * max(j - 1, 0)


def _chunk_q1(j):
    return min(B * (j + 2), T)


def _bank_writers():
    writers = [[] for _ in range(NBANK)]
    for j in range(NB):
        a, q1 = _chunk_q0(j), _chunk_q1(j)
        while a < q1:
            nxt = min(q1, (a // BANKQ + 1) * BANKQ)
            writers[a // BANKQ].append((j, a, nxt))
            a = nxt
    return writers


def build_nc(npair=NPAIR):
    nc = bacc.Bacc("TRN2", target_bir_lowering=False, debug=False)
    ncoup = npair // 2

    # qt is block-diagonal per couple: copy 0 = [Q_A^T; 0], copy 1 = [0; Q_B^T]
    # (partition-major layout [128, 2, T]).  The QK matmul then runs with
    # 128-partition moving data, which streams fp16 at 2 cols/cycle - twice
    # the rate of a 64-partition moving operand.  The couple-stacked kt
    # [K_A^T; K_B^T] is the shared 128-row stationary; the zero half of qt
    # kills the cross-pair terms.
    # copy-major qt: each [128, T] copy is a sequential DRAM read.  sg0..sg5
    # hold couple-0's critical kt/qt segments as separate fully-contiguous
    # tensors: column-sliced loads of the row-major tensors read 1-2KB rows
    # at 8KB stride and crawl at ~21GB/s, so every head-path DMA sources
    # from a contiguous block instead.
    # only the NONZERO 64 rows of each block-diagonal qt copy ship from
    # DRAM; the zero halves are memset on the otherwise-idle DVE.  That
    # halves the latency-critical input bytes.
    qt_d = nc.dram_tensor("qt", [ncoup, 2, D, T], QK_DT, kind="ExternalInput").ap()
    kt_d = nc.dram_tensor("kt", [ncoup, 2 * D, T], QK_DT, kind="ExternalInput").ap()
    sg_shapes = ((2 * D, 768), (D, 640), (2 * D, 768), (D, 1408),
                 (2 * D, 1280), (2 * D, 1280), (D, 2048))
    sg_d = []
    for i, shp in enumerate(sg_shapes):
        sg_d.append(nc.dram_tensor(f"sg{i}", list(shp), QK_DT,
                                   kind="ExternalInput").ap())
    va_d = nc.dram_tensor("va", [npair, B, NB * NAUG], AV_DT, kind="ExternalInput").ap()
    # unnormalized transposed output + Z row: [65, nbank, 512] fp16 per pair
    oz_d = nc.dram_tensor("oz", [npair, NAUG, NBANK * BANKQ], AV_DT,
                          kind="ExternalOutput").ap()

    Exp = mybir.ActivationFunctionType.Exp
    writers = _bank_writers()

    with tile.TileContext(nc) as tc:
        with (
            tc.tile_pool(name="qk", bufs=2) as qk_pool,
            tc.tile_pool(name="v", bufs=4) as v_pool,
            tc.tile_pool(name="e", bufs=2) as e_pool,
            tc.tile_pool(name="out", bufs=2) as out_pool,
            tc.tile_pool(name="qkps", bufs=2, space="PSUM") as qk_psum,
            tc.tile_pool(name="avps", bufs=4, space="PSUM") as av_psum,
        ):
            # ---- prologue: all input loads up front ----
            qts, kts, vas = [], [], []
            FQ = 512   # head segment: covers the first QK groups
            MQ = 1536
            for c in range(ncoup):
                qt_sb = qk_pool.tile([2 * D, 2, T], QK_DT, tag="qt")
                kt_sb = qk_pool.tile([2 * D, T], QK_DT, tag="kt")
                qts.append(qt_sb)
                kts.append(kt_sb)
            for ip in range(npair):
                va_sb = v_pool.tile([B, NB, NAUG], AV_DT, tag="va", name="va")
                vas.append(va_sb)
            # sync and scalar are HARDWARE DGE queues (fast); gpsimd DMA
            # is software-executed on the Q7 and ~2x slower, so couple-0's
            # critical stream rides sync/scalar from the contiguous sg
            # blocks and late-need bulk rides gpsimd as sequential full-copy
            # reads.
            # zero halves of the four qt copies (DVE is idle here)
            nc.vector.memset(qts[0][D:2 * D, 0, :], 0.0)
            nc.vector.memset(qts[0][0:D, 1, :], 0.0)
            nc.vector.memset(qts[1][D:2 * D, 0, :], 0.0)
            nc.vector.memset(qts[1][0:D, 1, :], 0.0)
            nc.sync.dma_start(out=kts[0][:, 0:768], in_=sg_d[0])
            nc.scalar.dma_start(out=qts[0][0:D, 0, 0:640], in_=sg_d[1])
            nc.sync.dma_start(out=kts[0][:, 768:MQ], in_=sg_d[2])
            nc.scalar.dma_start(out=qts[0][0:D, 0, 640:2048], in_=sg_d[3])
            nc.scalar.dma_start(out=qts[0][0:D, 0, 2048:T], in_=sg_d[6])
            nc.sync.dma_start(out=kts[0][:, MQ:2816], in_=sg_d[4])
            nc.scalar.dma_start(out=kts[0][:, 2816:T], in_=sg_d[5])
            nc.gpsimd.dma_start(out=vas[0], in_=va_d[0])
            nc.gpsimd.dma_start(out=qts[0][D:2 * D, 1, :], in_=qt_d[0, 1])
            nc.sync.dma_start(out=kts[1][:, 0:T], in_=kt_d[1])
            nc.gpsimd.dma_start(out=vas[1], in_=va_d[1])
            nc.gpsimd.dma_start(out=qts[1][0:D, 0, :], in_=qt_d[1, 0])
            nc.gpsimd.dma_start(out=qts[1][D:2 * D, 1, :], in_=qt_d[1, 1])
            nc.gpsimd.dma_start(out=vas[2], in_=va_d[2])
            nc.sync.dma_start(out=vas[3], in_=va_d[3])

            exps = [None] * npair
            osbs = [None] * npair

            # ---------- per-pair unit streams ----------
            def qk_group(ip, g):
                c, hh = ip // 2, ip % 2
                qt_sb, kt_sb = qts[c], kts[c]

                def run():
                    if g == 0:
                        exps[ip] = e_pool.tile([B, NB, 3 * B], AV_DT, tag="exp",
                                               name="exp")
                        osbs[ip] = out_pool.tile([NAUG, NBANK, BANKQ], AV_DT,
                                                 tag="osb", name="osb")
                    ps = qk_psum.tile([B, 2, BANKQ], F32, tag="qkps")
                    for ti in range(2):
                        j = 2 * g + ti
                        # uniform 384-wide window (edge chunks widened so
                        # every exp call is one full batch)
                        q0w = min(_chunk_q0(j), T - 3 * B)
                        nc.tensor.matmul(
                            ps[:, ti, 0:3 * B],
                            lhsT=kt_sb[:, j * B:(j + 1) * B],
                            rhs=qt_sb[:, hh, q0w:q0w + 3 * B],
                            start=True,
                            stop=True,
                        )
                    nc.scalar.activation(
                        out=exps[ip][:, 2 * g:2 * g + 2, :],
                        in_=ps[:, :, 0:3 * B],
                        func=Exp, scale=float(SCALE),
                    )
                return run

            def av_banks(ip, blist):
                # emit the banks' writers interleaved: consecutive matmuls
                # then hit DIFFERENT psum banks, so the accumulator
                # read-modify-write of one bank overlaps the other's stream
                def run():
                    exp_sb = exps[ip]
                    va_sb = vas[ip]
                    avs, wls = [], []
                    for b in blist:
                        avs.append(av_psum.tile([NAUG, BANKQ], F32, tag="avps",
                                                name="avtile"))
                        wls.append(list(writers[b]))
                    for wi in range(max(len(w) for w in wls)):
                        for k, b in enumerate(blist):
                            if wi >= len(wls[k]):
                                continue
                            j, a0, a1 = wls[k][wi]
                            q0w = min(_chunk_q0(j), T - 3 * B)
                            nc.tensor.matmul(
                                avs[k][:, a0 - BANKQ * b:a1 - BANKQ * b],
                                lhsT=va_sb[:, j, :],
                                rhs=exp_sb[:, j, a0 - q0w:a1 - q0w],
                                start=(wi == 0),  # zeroes the whole 2KB bank
                                stop=(wi == len(wls[k]) - 1),
                                skip_group_check=(wi != 0),
                            )
                    for k, b in enumerate(blist):
                        # eviction on DVE (gpsimd cannot access PSUM), then a
                        # per-bank store so the output drains continuously.
                        # The very last bank evicts on ACT (idle after its
                        # final exp) so the tail's two evictions run in
                        # parallel instead of serializing on DVE.
                        if ip == npair - 1 and b == NBANK - 1:
                            nc.scalar.activation(
                                out=osbs[ip][:, b, :], in_=avs[k],
                                func=mybir.ActivationFunctionType.Copy)
                        else:
                            nc.vector.tensor_copy(out=osbs[ip][:, b, :],
                                                  in_=avs[k])
                        # all stores on the hardware sync queue: exec time
                        # ends at the last DMA + queue drains, and the
                        # software gpsimd queue both transfers slowly and
                        # drains slowly when it still has pending stores
                        seng = nc.sync
                        seng.dma_start(
                            out=oz_d[ip, :, b * BANKQ:(b + 1) * BANKQ],
                            in_=osbs[ip][:, b, :],
                        )
                return run

            # ---------- emission: AV banks woven into the QK stream ----------
            # bank (p, b) consumes chunks up to 4b+6, i.e. QK group 2b+3 of
            # pair p.  Emit it SLACK groups later so the exp it needs is
            # already drained from ACT and the PE never stalls mid-stream;
            # late banks spill into the next pair's groups.
            # bank-pair (b0, b0+1) consumes chunks up to 4(b0+1)+6, i.e. QK
            # group 2*b0+5 of its pair.  The first pair runs with less slack
            # (no AV backlog exists yet to fill PE waits anyway) and the
            # last pair emits single banks as soon as their exps exist so
            # the tail after the final QK group is one bank, not four.
            NG = NB // 2
            av_ready = []
            for p in range(npair):
                slack = 2 if p == 0 else (3 if p < npair - 1 else 1)
                if p < npair - 1:
                    for b0 in range(0, NBANK, 2):
                        av_ready.append(
                            (NG * p + min(2 * b0 + 5, NG - 1) + slack,
                             p, (b0, b0 + 1)))
                else:
                    for b in range(NBANK):
                        av_ready.append(
                            (NG * p + min(2 * b + 3, NG - 1) + slack,
                             p, (b,)))
            av_ready.sort(key=lambda t: t[0])
            ai = 0
            for gi in range(npair * NG):
                qk_group(gi // NG, gi % NG)()
                while ai < len(av_ready) and av_ready[ai][0] <= gi:
                    _, p, blist = av_ready[ai]
                    av_banks(p, blist)()
                    ai += 1
            while ai < len(av_ready):
                _, p, blist = av_ready[ai]
                av_banks(p, blist)()
                ai += 1

    nc.compile()
    return nc


_CACHE = {}


def _prep_core(q, k, v, core):
    sl = slice(core * NPAIR, (core + 1) * NPAIR)
    np_qk = mybir.dt.np(QK_DT)
    qs, ks, vs = q[sl], k[sl], v[sl]
    ncoup = NPAIR // 2
    # kt: [ncoup, 2D, T] - two pairs of a couple stacked on partitions
    # qt: [ncoup, 2D, 2, T] block-diagonal: [:, 0:64, 0, :] = Q_A^T,
    #     [:, 64:128, 1, :] = Q_B^T, rest zeros
    # nonzero halves only: copy 0 = Q_A^T rows, copy 1 = Q_B^T rows
    qt = np.ascontiguousarray(
        qs.reshape(ncoup, 2, T, D).transpose(0, 1, 3, 2).astype(np_qk))
    kt = np.ascontiguousarray(
        ks.reshape(ncoup, 2, T, D).transpose(0, 1, 3, 2)
        .reshape(ncoup, 2 * D, T).astype(np_qk))
    # va: [npair, B, NB*NAUG] kpos-major with ones column
    va = np.concatenate([vs, np.ones((NPAIR, T, 1), np.float32)], axis=-1)
    va = va.reshape(NPAIR, NB, B, NAUG).transpose(0, 2, 1, 3)
    va = np.ascontiguousarray(
        va.reshape(NPAIR, B, NB * NAUG).astype(mybir.dt.np(AV_DT))
    )
    out = {"qt": qt, "kt": kt, "va": va}
    for i, (src_t, c0, c1) in enumerate((
            (kt[0], 0, 768), (qt[0, 0], 0, 640),
            (kt[0], 768, 1536), (qt[0, 0], 640, 2048),
            (kt[0], 1536, 2816), (kt[0], 2816, 4096),
            (qt[0, 0], 2048, 4096))):
        out[f"sg{i}"] = np.ascontiguousarray(src_t[:, c0:c1])
    return out


def kernel(query_layer, key_layer, value_layer, attention_mask):
    q = np.asarray(query_layer, np.float32).reshape(N * H, T, D)
    k = np.asarray(key_layer, np.float32).reshape(N * H, T, D)
    v = np.asarray(value_layer, np.float32).reshape(N * H, T, D)

    if "nc" not in _CACHE:
        _CACHE["nc"] = build_nc()
    nc = _CACHE["nc"]

    in_maps = [_prep_core(q, k, v, core) for core in range(NCORES)]
    res = run_bass_kernel_spmd(nc, in_maps, core_ids=list(range(NCORES)))
    # [NCORES, NPAIR, 65, NBANK*BANKQ] fp16 -> [32, 65, 4096] f32
    oz = np.stack([r["oz"] for r in res.results]).astype(np.float32)
    oz = oz.reshape(N * H, NAUG, T)
    o_un = oz[:, 0:D, :]              # [32, 64, 4096] unnormalized out^T
    z = oz[:, D, :]                   # [32, 4096]

    # global-token rank-1 term for queries >= 2 blocks (blocks 0-1 already
    # include kpos 0 through their local window)
    eg = np.exp(np.einsum('ptd,pd->pt', q, k[:, 0]) * SCALE)  # [32, 4096]
    o_un[:, :, 2 * B:] += eg[:, None, 2 * B:] * v[:, 0, :, None]
    z[:, 2 * B:] += eg[:, 2 * B:]

    out = (o_un / z[:, None, :]).transpose(0, 2, 1)  # [32, 4096, 64]

    # global query row: exact softmax over all positions
    p0 = np.exp(np.einsum('pd,ptd->pt', q[:, 0], k) * SCALE)
    out[:, 0, :] = np.einsum('pt,ptd->pd', p0, v) / p0.sum(1)[:, None]

    return np.ascontiguousarray(out.reshape(N, H, T, D).astype(np.float32))
